# revision 8
# baseline (speedup 1.0000x reference)
"""Sliding-window causal GQA attention (RoPE) for Trainium2, 8-core SPMD.

Problem: x:(4,2048,2048), Wq:(2048,2048), Wk/Wv:(512,2048), Wo:(2048,2048)
  q = rope(x @ Wq.T) 16 heads, k/v = (x @ Wk.T / x @ Wv.T) 4 kv heads (GQA x4),
  causal sliding-window attention (W=1024), out = z @ Wo.T.

Sharding: 8 cores = 4 batches x 2 head-groups (8 q-heads / 2 kv-heads each).
Each core computes a partial output (its head-group's Wo contribution) for its
batch; host sums the two partials per batch.

Per-core kernel, all matmuls bf16 (full-rate on the PE array):
  - super-major loop: one query super (256 queries) at a time; K/V/Q
    projections for the super, then attention for all 8 heads, with the
    output projection for the PREVIOUS super interleaved so z never leaves
    SBUF (no DRAM spill) and the tensor engine never waits on a phase
    boundary.
  - layout: qT/kT as (head_dim, L) ["transposed"], v as (L, head_dim)
  - scores computed transposed S.T (keys on partitions, queries free) so the
    softmax denominator comes from a ones-vector matmul (row form) and P.T
    feeds the PV matmul directly with no on-chip transposes.
  - no max-subtraction in softmax: logits are O(1) here, exp is safe.
  - sliding window at 128-block granularity: query-super of 256 x up to 10
    key-blocks; boundary blocks masked via precomputed 0/1 tiles.
  - weights SBUF-resident (loaded once); x streamed per super in bf16.
"""

import math
import numpy as np

H = 16
D = 4
WINDOW = 1024
THETA = 10000.0
N, L, E = 4, 2048, 2048
P = 128
DH = E // H          # 128 head dim
NH = H // 2          # 8 q heads per core
NKV = 2              # kv heads per core
NB = L // P          # 16 key blocks
NKT = E // P         # 16 contraction tiles
NS = L // 256        # 8 query supers
SCALE = 1.0 / math.sqrt(DH)

_NC = None


def _kbs_for_super(t):
    """Key blocks overlapping the window of query super t (256 queries)."""
    return list(range(max(0, 2 * t - 8), 2 * t + 2))


def build_nc():
    from contextlib import ExitStack
    from concourse import bacc, tile, mybir

    F32 = mybir.dt.float32
    BF16 = mybir.dt.bfloat16
    EXP = mybir.ActivationFunctionType.Exp

    SHUF_SWAP = [i ^ 1 for i in range(32)]

    nc = bacc.Bacc("TRN2", target_bir_lowering=False, debug=False)
    # prepacked inputs (see _pack_core_inputs for layouts)
    xq = nc.dram_tensor("xq", [P, NS * NKT * 256], BF16, kind="ExternalInput").ap()
    wqp = nc.dram_tensor("wqp", [NH * P, NKT * DH], BF16, kind="ExternalInput").ap()
    wkv = nc.dram_tensor("wkv", [P, NKT * 512], BF16, kind="ExternalInput").ap()
    woT = nc.dram_tensor("woT", [NH * P, E], BF16, kind="ExternalInput").ap()
    cosT = nc.dram_tensor("cosT", [P, L], F32, kind="ExternalInput").ap()
    sinT = nc.dram_tensor("sinT", [P, L], F32, kind="ExternalInput").ap()
    masks = nc.dram_tensor("masks", [4 * P, 256], BF16, kind="ExternalInput").ap()
    out = nc.dram_tensor("out", [L, E], F32, kind="ExternalOutput").ap()

    with tile.TileContext(nc) as tc, ExitStack() as stk:
        const = stk.enter_context(tc.tile_pool(name="const", bufs=1))
        ones_f = const.tile([P, 1], F32, tag="ones_f")
        nc.vector.memset(ones_f[:], 1.0)
        onesrow_f = const.tile([1, P], F32, tag="onesrow_f")
        nc.vector.memset(onesrow_f[:], 1.0)
        ones = const.tile([P, 1], BF16, tag="ones")
        nc.vector.tensor_copy(ones[:], ones_f[:])
        onesrow = const.tile([1, P], BF16, tag="onesrow")
        nc.vector.tensor_copy(onesrow[:], onesrow_f[:])

        resid = stk.enter_context(tc.tile_pool(name="resid", bufs=1))
        # resident weights: emitted in consumption order; first-super tiles
        # come first so compute starts as soon as possible.
        kvw = resid.tile([P, NKT, 512], BF16, tag="kvw")
        for dc in range(4):
            nc.sync.dma_start(out=kvw[:, 4 * dc:4 * (dc + 1), :],
                              in_=wkv[:, dc * 2048:(dc + 1) * 2048])
        wq = [resid.tile([P, NKT, DH], BF16, tag=f"wq{h}", name=f"wq{h}")
              for h in range(NH)]
        for h in range(NH):
            nc.sync.dma_start(out=wq[h][:], in_=wqp[h * P:(h + 1) * P, :])
        # mask kinds: 0=diagA (k<=q), 1=diagB (k<=q-128),
        #             2=farA (k>=q+1), 3=farB (k>=q-127)
        mk = [const.tile([P, 256], BF16, tag=f"mk{i}", name=f"mk{i}") for i in range(4)]
        for i in range(4):
            nc.sync.dma_start(out=mk[i][:], in_=masks[i * P:(i + 1) * P, :])
        cos_r = resid.tile([P, L], F32, tag="cos_r")
        sin_r = resid.tile([P, L], F32, tag="sin_r")
        for dc in range(4):
            nc.sync.dma_start(out=cos_r[:, dc * 512:(dc + 1) * 512],
                              in_=cosT[:, dc * 512:(dc + 1) * 512])
            nc.sync.dma_start(out=sin_r[:, dc * 512:(dc + 1) * 512],
                              in_=sinT[:, dc * 512:(dc + 1) * 512])
        wo = [resid.tile([P, E], BF16, tag=f"wo{h}", name=f"wo{h}") for h in range(NH)]
        for h in range(NH):
            nc.sync.dma_start(out=wo[h][:], in_=woT[h * P:(h + 1) * P, :])

        kT = [resid.tile([P, L], BF16, tag=f"kT{i}", name=f"kT{i}") for i in range(NKV)]
        vt = [[resid.tile([P, P], BF16, tag=f"v{i}_{b}", name=f"v{i}_{b}")
               for b in range(NB)] for i in range(NKV)]

        def rope_evict(dest, psum, cos_sl, sin_sl, tmp_pool):
            # dest = psum * cos + pairswap(psum) * sin   (sin pre-signed)
            tmp = tmp_pool.tile([P, 256], F32, tag="ropetmp", name="ropetmp")
            nc.vector.stream_shuffle(tmp[:], psum, SHUF_SWAP)
            nc.vector.tensor_mul(tmp[:], tmp[:], sin_sl)
            nc.vector.tensor_mul(dest, psum, cos_sl)
            nc.vector.tensor_add(dest, dest, tmp[:])

        osb = stk.enter_context(tc.tile_pool(name="osb", bufs=3))
        pp = stk.enter_context(tc.tile_pool(name="pp", bufs=2, space="PSUM"))
        pop = stk.enter_context(tc.tile_pool(name="po", bufs=2, space="PSUM"))
        psp = stk.enter_context(tc.tile_pool(name="ps", bufs=2, space="PSUM"))
        pzp = stk.enter_context(tc.tile_pool(name="pz", bufs=1, space="PSUM"))
        pbp = stk.enter_context(tc.tile_pool(name="pb", bufs=1, space="PSUM"))

        def oproj(t, zev_of):
            # output projection for super t: out[256q x 2048e] partial
            for qh in range(2):
                for ec in range(4):
                    po = pop.tile([P, 512], F32, tag="po")
                    for h in range(NH):
                        nc.tensor.matmul(
                            po[:],
                            zev_of[h][:, qh * P:(qh + 1) * P],
                            wo[h][:, ec * 512:(ec + 1) * 512],
                            start=(h == 0), stop=(h == NH - 1),
                        )
                    ot = osb.tile([P, 512], F32, tag="ot")
                    nc.scalar.copy(ot[:], po[:])
                    nc.sync.dma_start(
                        out=out[t * 256 + qh * P: t * 256 + (qh + 1) * P,
                                ec * 512:(ec + 1) * 512],
                        in_=ot[:])

        with tc.tile_pool(name="xt", bufs=2) as xpool, \
             tc.tile_pool(name="work", bufs=3) as work, \
             tc.tile_pool(name="qt", bufs=2) as qtpool, \
             tc.tile_pool(name="zev", bufs=2) as zevpool, \
             tc.tile_pool(name="rtmp", bufs=2) as rtmp:
            zev_prev = None
            for t in range(NS):
                c0 = 256 * t
                xt = xpool.tile([P, NKT, 256], BF16, tag="xt")
                for dc in range(4):
                    nc.sync.dma_start(
                        out=xt[:, 4 * dc:4 * (dc + 1), :],
                        in_=xq[:, t * NKT * 256 + dc * 1024:
                               t * NKT * 256 + (dc + 1) * 1024])
                cos_q = cos_r[:, c0:c0 + 256]
                sin_q = sin_r[:, c0:c0 + 256]

                # K projection (+RoPE) for both kv heads
                for kv in range(NKV):
                    pk = pp.tile([P, 256], F32, tag="pp")
                    for kt in range(NKT):
                        nc.tensor.matmul(
                            pk[:],
                            kvw[:, kt, kv * DH:(kv + 1) * DH],
                            xt[:, kt, :],
                            start=(kt == 0), stop=(kt == NKT - 1),
                        )
                    rope_evict(kT[kv][:, c0:c0 + 256], pk[:], cos_q, sin_q, rtmp)

                # V projection (both kv heads at once, natural layout)
                for lb in range(2):
                    b = 2 * t + lb
                    pv = pp.tile([P, 256], F32, tag="pp")
                    for kt in range(NKT):
                        nc.tensor.matmul(
                            pv[:],
                            xt[:, kt, lb * P:(lb + 1) * P],
                            kvw[:, kt, 256:512],
                            start=(kt == 0), stop=(kt == NKT - 1),
                        )
                    for kv in range(NKV):
                        nc.scalar.copy(vt[kv][b][:], pv[:, kv * DH:(kv + 1) * DH])

                # Q projection + RoPE for all heads
                qth = []
                for h in range(NH):
                    pq = pp.tile([P, 256], F32, tag="pp")
                    for kt in range(NKT):
                        nc.tensor.matmul(
                            pq[:],
                            wq[h][:, kt, :],
                            xt[:, kt, :],
                            start=(kt == 0), stop=(kt == NKT - 1),
                        )
                    qt = qtpool.tile([P, 256], BF16, tag=f"qt{h}", name=f"qt{h}")
                    rope_evict(qt[:], pq[:], cos_q, sin_q, rtmp)
                    qth.append(qt)
                    if h == 3 and zev_prev is not None:
                        oproj(t - 1, zev_prev)

                # attention for each head
                kbs = _kbs_for_super(t)
                nkb = len(kbs)
                zev_cur = []
                for h in range(NH):
                    kv = h // (NH // NKV)
                    pt = work.tile([P, 10, 256], BF16, tag="pt")
                    # scores (transposed: keys on partitions) in chunks of 2 kb
                    for ci in range(0, nkb, 2):
                        ps = psp.tile([P, 512], F32, tag="ps")
                        for i in range(2):
                            kb = kbs[ci + i]
                            nc.tensor.matmul(
                                ps[:, i * 256:(i + 1) * 256],
                                kT[kv][:, kb * P:(kb + 1) * P],
                                qth[h][:],
                                start=True, stop=True,
                            )
                        nc.scalar.activation(
                            pt[:, ci:ci + 2, :], ps[:], EXP, scale=SCALE)
                    # window masks on boundary blocks
                    for i, kb in enumerate(kbs):
                        kind = None
                        if kb == 2 * t:
                            kind = 0
                        elif kb == 2 * t + 1:
                            kind = 1
                        elif kb == 2 * t - 8:
                            kind = 2
                        elif kb == 2 * t - 7:
                            kind = 3
                        if kind is not None:
                            sl = pt[:, i, :]
                            nc.vector.tensor_mul(sl, sl, mk[kind][:])
                    # denominator (ones matmul) + PV, accumulated over kbs
                    pzt = pzp.tile([P, 256], F32, tag="pz", name="pzt")
                    sut = pbp.tile([1, 256], F32, tag="su", name="sut")
                    pz = pzt[:, :]
                    su = sut[:, :]
                    for i, kb in enumerate(kbs):
                        st, sp = (i == 0), (i == nkb - 1)
                        nc.tensor.matmul(
                            su, ones[:], pt[:, i, :], start=st, stop=sp)
                        nc.tensor.matmul(
                            pz, vt[kv][kb][:], pt[:, i, :], start=st, stop=sp)
                    # normalize: bcast sums across partitions (K=1 matmul),
                    # full-lane approx reciprocal, multiply.
                    sus = work.tile([1, 256], BF16, tag="sus")
                    nc.vector.tensor_copy(sus[:], su)
                    bcps = psp.tile([P, 512], F32, tag="ps")
                    nc.tensor.matmul(bcps[:, 0:256], onesrow[:], sus[:],
                                     start=True, stop=True)
                    rec = work.tile([P, 256], F32, tag="rec")
                    nc.vector.reciprocal_approx_fast(rec[:], bcps[:, 0:256])
                    zev = zevpool.tile([P, 256], BF16, tag=f"zev{h}", name=f"zev{h}")
                    nc.vector.tensor_mul(zev[:], pz, rec[:])
                    zev_cur.append(zev)
                zev_prev = zev_cur
            oproj(NS - 1, zev_prev)

    nc.compile()
    return nc


def _host_tables():
    freqs = 1.0 / (THETA ** (np.arange(0, DH - 1, 2, dtype=np.float64) / DH))
    ang = np.arange(L, dtype=np.float64)[:, None] * freqs[None, :]  # (L, 64)
    cos = np.cos(ang)
    sin = np.sin(ang)
    cosT = np.empty((P, L), np.float32)
    sinT = np.empty((P, L), np.float32)
    cosT[0::2, :] = cos.T
    cosT[1::2, :] = cos.T
    sinT[0::2, :] = -sin.T
    sinT[1::2, :] = sin.T
    return cosT, sinT


def _host_masks():
    import ml_dtypes
    k = np.arange(P)[:, None]
    q = np.arange(256)[None, :]
    m = np.stack([
        (k <= q), (k <= q - 128), (k >= q + 1), (k >= q - 127),
    ]).astype(ml_dtypes.bfloat16)
    return m.reshape(4 * P, 256)


def _pack_core_inputs(x, Wq, Wk, Wv, Wo, n, g):
    """Prepacked per-core inputs (bf16); long contiguous per-partition runs."""
    import ml_dtypes
    BF = ml_dtypes.bfloat16
    xT = np.ascontiguousarray(x[n].T)                      # (E, L)
    # xq[p, t*NKT*256 + kt*256 + c] = xT[kt*128+p, 256t + c]
    xqs = xT.reshape(NKT, P, NS, 256).transpose(1, 2, 0, 3).reshape(P, NS * NKT * 256)
    # wqp[h*128+p, kt*128+c] = Wq.T[kt*128+p, g*1024+h*128+c]
    wqT = Wq[g * 1024:(g + 1) * 1024, :].T                 # (E, 1024)
    wqp = wqT.reshape(NKT, P, NH, DH).transpose(2, 1, 0, 3).reshape(NH * P, NKT * DH)
    # wkv[p, kt*512+j]: j<256 -> Wk.T slice, j>=256 -> Wv.T slice
    wkT = Wk[g * 256:(g + 1) * 256, :].T.reshape(NKT, P, 256)
    wvT = Wv[g * 256:(g + 1) * 256, :].T.reshape(NKT, P, 256)
    wkvp = np.concatenate([wkT, wvT], axis=2)              # (kt, p, 512)
    wkvp = wkvp.transpose(1, 0, 2).reshape(P, NKT * 512)
    woT = Wo[:, g * 1024:(g + 1) * 1024].T                 # (1024, E)
    return {
        "xq": np.ascontiguousarray(xqs).astype(BF),
        "wqp": np.ascontiguousarray(wqp).astype(BF),
        "wkv": np.ascontiguousarray(wkvp).astype(BF),
        "woT": np.ascontiguousarray(woT).astype(BF),
    }


def make_in_maps(inputs):
    x = np.asarray(inputs["x"], np.float32)
    Wq = np.asarray(inputs["Wq"], np.float32)
    Wk = np.asarray(inputs["Wk"], np.float32)
    Wv = np.asarray(inputs["Wv"], np.float32)
    Wo = np.asarray(inputs["Wo"], np.float32)
    cosT, sinT = _host_tables()
    masks = _host_masks()
    in_maps = []
    for c in range(8):
        n, g = c % 4, c // 4
        m = _pack_core_inputs(x, Wq, Wk, Wv, Wo, n, g)
        m.update({"cosT": cosT, "sinT": sinT, "masks": masks})
        in_maps.append(m)
    return in_maps


def kernel(x, Wq, Wk, Wv, Wo):
    global _NC
    if _NC is None:
        _NC = build_nc()
    nc = _NC

    in_maps = make_in_maps(dict(x=x, Wq=Wq, Wk=Wk, Wv=Wv, Wo=Wo))
    from concourse.bass_utils import run_bass_kernel_spmd
    res = run_bass_kernel_spmd(nc, in_maps, list(range(8)), trace=False)
    out = np.empty((N, L, E), np.float32)
    for n_ in range(4):
        out[n_] = res.results[n_]["out"] + res.results[4 + n_]["out"]
    return out


if __name__ == "__main__":
    rng = np.random.default_rng(0)
    x = rng.standard_normal((N, L, E), dtype=np.float32)
    Wq = (rng.standard_normal((E, E), dtype=np.float32) * 0.02)
    Wk = (rng.standard_normal((E // D, E), dtype=np.float32) * 0.02)
    Wv = (rng.standard_normal((E // D, E), dtype=np.float32) * 0.02)
    Wo = (rng.standard_normal((E, E), dtype=np.float32) * 0.02)
    print(kernel(x, Wq, Wk, Wv, Wo).shape)


# revision 14
# speedup vs baseline: 1.0191x; 1.0191x over previous
"""Sliding-window causal GQA attention (RoPE) for Trainium2, 8-core SPMD.

Problem: x:(4,2048,2048), Wq:(2048,2048), Wk/Wv:(512,2048), Wo:(2048,2048)
  q = rope(x @ Wq.T) 16 heads, k/v = (x @ Wk.T / x @ Wv.T) 4 kv heads (GQA x4),
  causal sliding-window attention (W=1024), out = z @ Wo.T.

Sharding: 8 cores = 4 batches x 2 head-groups (8 q-heads / 2 kv-heads each).
Each core computes a partial output (its head-group's Wo contribution) for its
batch; host sums the two partials per batch.

Per-core kernel, all matmuls bf16 (full-rate on the PE array):
  - super-major loop: one query super (256 queries) at a time; K/V/Q
    projections for the super, then attention for all 8 heads, with the
    output projection for the PREVIOUS super interleaved so z never leaves
    SBUF (no DRAM spill) and the tensor engine never waits on a phase
    boundary.
  - layout: qT/kT as (head_dim, L) ["transposed"], v as (L, head_dim)
  - scores computed transposed S.T (keys on partitions, queries free) so the
    softmax denominator comes from a ones-vector matmul (row form) and P.T
    feeds the PV matmul directly with no on-chip transposes.
  - no max-subtraction in softmax: logits are O(1) here, exp is safe.
  - sliding window at 128-block granularity: query-super of 256 x up to 10
    key-blocks; boundary blocks masked via precomputed 0/1 tiles.
  - weights SBUF-resident (loaded once); x streamed per super in bf16.
"""

import math
import numpy as np

H = 16
D = 4
WINDOW = 1024
THETA = 10000.0
N, L, E = 4, 2048, 2048
P = 128
DH = E // H          # 128 head dim
NH = H // 2          # 8 q heads per core
NKV = 2              # kv heads per core
NB = L // P          # 16 key blocks
NKT = E // P         # 16 contraction tiles
NS = L // 256        # 8 query supers
SCALE = 1.0 / math.sqrt(DH)

_NC = None


def _kbs_for_super(t):
    """Key blocks overlapping the window of query super t (256 queries)."""
    return list(range(max(0, 2 * t - 8), 2 * t + 2))


def build_nc():
    from contextlib import ExitStack
    from concourse import bacc, tile, mybir

    F32 = mybir.dt.float32
    BF16 = mybir.dt.bfloat16
    EXP = mybir.ActivationFunctionType.Exp

    SHUF_SWAP = [i ^ 1 for i in range(32)]

    nc = bacc.Bacc("TRN2", target_bir_lowering=False, debug=False)
    # prepacked inputs (see _pack_core_inputs for layouts)
    xq = nc.dram_tensor("xq", [P, NS * NKT * 256], BF16, kind="ExternalInput").ap()
    wqp = nc.dram_tensor("wqp", [NH * P, NKT * DH], BF16, kind="ExternalInput").ap()
    wkv = nc.dram_tensor("wkv", [P, NKT * 512], BF16, kind="ExternalInput").ap()
    woT = nc.dram_tensor("woT", [NH * P, E], BF16, kind="ExternalInput").ap()
    cosT = nc.dram_tensor("cosT", [P, L], F32, kind="ExternalInput").ap()
    sinT = nc.dram_tensor("sinT", [P, L], F32, kind="ExternalInput").ap()
    masks = nc.dram_tensor("masks", [4 * P, 256], BF16, kind="ExternalInput").ap()
    out = nc.dram_tensor("out", [L, E], F32, kind="ExternalOutput").ap()

    with tile.TileContext(nc) as tc, ExitStack() as stk:
        const = stk.enter_context(tc.tile_pool(name="const", bufs=1))
        ones_f = const.tile([P, 1], F32, tag="ones_f")
        nc.vector.memset(ones_f[:], 1.0)
        onesrow_f = const.tile([1, P], F32, tag="onesrow_f")
        nc.vector.memset(onesrow_f[:], 1.0)
        ones = const.tile([P, 1], BF16, tag="ones")
        nc.vector.tensor_copy(ones[:], ones_f[:])
        onesrow = const.tile([1, P], BF16, tag="onesrow")
        nc.vector.tensor_copy(onesrow[:], onesrow_f[:])

        resid = stk.enter_context(tc.tile_pool(name="resid", bufs=1))
        xpool = stk.enter_context(tc.tile_pool(name="xt", bufs=2))
        # DMA emission in consumption order: x for super 0 first, then the
        # weights in the order compute needs them, so the PE starts ASAP.
        xts = []
        xt0 = xpool.tile([P, NKT, 256], BF16, tag="xt", name="xt0")
        for dc in range(4):
            nc.sync.dma_start(out=xt0[:, 4 * dc:4 * (dc + 1), :],
                              in_=xq[:, dc * 1024:(dc + 1) * 1024])
        xts.append(xt0)
        kvw = resid.tile([P, NKT, 512], BF16, tag="kvw")
        for dc in range(4):
            nc.sync.dma_start(out=kvw[:, 4 * dc:4 * (dc + 1), :],
                              in_=wkv[:, dc * 2048:(dc + 1) * 2048])
        cos_r = resid.tile([P, L], F32, tag="cos_r")
        sin_r = resid.tile([P, L], F32, tag="sin_r")
        for dc in range(4):
            nc.sync.dma_start(out=cos_r[:, dc * 512:(dc + 1) * 512],
                              in_=cosT[:, dc * 512:(dc + 1) * 512])
            nc.sync.dma_start(out=sin_r[:, dc * 512:(dc + 1) * 512],
                              in_=sinT[:, dc * 512:(dc + 1) * 512])
        wq = [resid.tile([P, NKT, DH], BF16, tag=f"wq{h}", name=f"wq{h}")
              for h in range(NH)]
        for h in range(NH):
            nc.sync.dma_start(out=wq[h][:], in_=wqp[h * P:(h + 1) * P, :])
        # mask kinds: 0=diagA (k<=q), 1=diagB (k<=q-128),
        #             2=farA (k>=q+1), 3=farB (k>=q-127)
        mk = [const.tile([P, 256], BF16, tag=f"mk{i}", name=f"mk{i}") for i in range(4)]
        for i in range(4):
            nc.sync.dma_start(out=mk[i][:], in_=masks[i * P:(i + 1) * P, :])
        wo = [resid.tile([P, E], BF16, tag=f"wo{h}", name=f"wo{h}") for h in range(NH)]
        for h in range(NH):
            nc.sync.dma_start(out=wo[h][:], in_=woT[h * P:(h + 1) * P, :])

        kT = [resid.tile([P, L], BF16, tag=f"kT{i}", name=f"kT{i}") for i in range(NKV)]
        vt = [[resid.tile([P, P], BF16, tag=f"v{i}_{b}", name=f"v{i}_{b}")
               for b in range(NB)] for i in range(NKV)]

        def rope_evict(dest, psum, cos_sl, sin_sl, tmp_pool):
            # dest = psum * cos + pairswap(psum) * sin   (sin pre-signed)
            # ACT-copy the psum out first so the PSUM bank frees in one op
            # instead of being held across the whole DVE rope chain.
            cp = tmp_pool.tile([P, 256], F32, tag="ropecp", name="ropecp")
            nc.scalar.copy(cp[:], psum)
            tmp = tmp_pool.tile([P, 256], F32, tag="ropetmp", name="ropetmp")
            nc.vector.stream_shuffle(tmp[:], cp[:], SHUF_SWAP)
            nc.vector.tensor_mul(tmp[:], tmp[:], sin_sl)
            nc.vector.tensor_mul(dest, cp[:], cos_sl)
            nc.vector.tensor_add(dest, dest, tmp[:])

        osb = stk.enter_context(tc.tile_pool(name="osb", bufs=3))
        pp = stk.enter_context(tc.tile_pool(name="pp", bufs=2, space="PSUM"))
        pop = stk.enter_context(tc.tile_pool(name="po", bufs=2, space="PSUM"))
        psp = stk.enter_context(tc.tile_pool(name="ps", bufs=2, space="PSUM"))
        pzp = stk.enter_context(tc.tile_pool(name="pz", bufs=1, space="PSUM"))
        pbp = stk.enter_context(tc.tile_pool(name="pb", bufs=1, space="PSUM"))

        def oproj(t, zev_of):
            # output projection for super t: out[256q x 2048e] partial
            for qh in range(2):
                for ec in range(4):
                    po = pop.tile([P, 512], F32, tag="po")
                    for h in range(NH):
                        nc.tensor.matmul(
                            po[:],
                            zev_of[h][:, qh * P:(qh + 1) * P],
                            wo[h][:, ec * 512:(ec + 1) * 512],
                            start=(h == 0), stop=(h == NH - 1),
                        )
                    ot = osb.tile([P, 512], F32, tag="ot")
                    nc.scalar.copy(ot[:], po[:])
                    nc.sync.dma_start(
                        out=out[t * 256 + qh * P: t * 256 + (qh + 1) * P,
                                ec * 512:(ec + 1) * 512],
                        in_=ot[:])

        with tc.tile_pool(name="work", bufs=3) as work, \
             tc.tile_pool(name="qt", bufs=2) as qtpool, \
             tc.tile_pool(name="zev", bufs=2) as zevpool, \
             tc.tile_pool(name="rtmp", bufs=2) as rtmp:
            zev_prev = None
            for t in range(NS):
                c0 = 256 * t
                xt = xts[t]
                if t + 1 < NS:
                    # prefetch next super's x
                    xn = xpool.tile([P, NKT, 256], BF16, tag="xt", name="xtn")
                    for dc in range(4):
                        nc.sync.dma_start(
                            out=xn[:, 4 * dc:4 * (dc + 1), :],
                            in_=xq[:, (t + 1) * NKT * 256 + dc * 1024:
                                   (t + 1) * NKT * 256 + (dc + 1) * 1024])
                    xts.append(xn)
                cos_q = cos_r[:, c0:c0 + 256]
                sin_q = sin_r[:, c0:c0 + 256]

                # Projection chains run in interleaved pairs on two PSUM
                # banks: same-bank back-to-back accumulation serializes at
                # ~200ns, alternating banks pipelines at full rate.

                # K projection (+RoPE) for both kv heads
                pks = [pp.tile([P, 256], F32, tag="pp", name=f"pk{kv}")
                       for kv in range(NKV)]
                for kt in range(NKT):
                    for kv in range(NKV):
                        nc.tensor.matmul(
                            pks[kv][:],
                            kvw[:, kt, kv * DH:(kv + 1) * DH],
                            xt[:, kt, :],
                            start=(kt == 0), stop=(kt == NKT - 1),
                        )
                for kv in range(NKV):
                    rope_evict(kT[kv][:, c0:c0 + 256], pks[kv][:], cos_q, sin_q, rtmp)

                # V projection (both kv heads at once, natural layout)
                pvs = [pp.tile([P, 256], F32, tag="pp", name=f"pv{lb}")
                       for lb in range(2)]
                for kt in range(NKT):
                    for lb in range(2):
                        nc.tensor.matmul(
                            pvs[lb][:],
                            xt[:, kt, lb * P:(lb + 1) * P],
                            kvw[:, kt, 256:512],
                            start=(kt == 0), stop=(kt == NKT - 1),
                        )
                for lb in range(2):
                    b = 2 * t + lb
                    for kv in range(NKV):
                        nc.scalar.copy(vt[kv][b][:], pvs[lb][:, kv * DH:(kv + 1) * DH])

                # Q projection + RoPE, interleaved head pairs
                qth = []
                for hp in range(NH // 2):
                    pqs = [pp.tile([P, 256], F32, tag="pp", name=f"pq{j}")
                           for j in range(2)]
                    for kt in range(NKT):
                        for j in range(2):
                            nc.tensor.matmul(
                                pqs[j][:],
                                wq[2 * hp + j][:, kt, :],
                                xt[:, kt, :],
                                start=(kt == 0), stop=(kt == NKT - 1),
                            )
                    for j in range(2):
                        h = 2 * hp + j
                        qt = qtpool.tile([P, 256], BF16, tag=f"qt{h}", name=f"qt{h}")
                        rope_evict(qt[:], pqs[j][:], cos_q, sin_q, rtmp)
                        qth.append(qt)
                    if hp == 1 and zev_prev is not None:
                        oproj(t - 1, zev_prev)

                # attention for each head
                kbs = _kbs_for_super(t)
                nkb = len(kbs)
                zev_cur = []
                for h in range(NH):
                    kv = h // (NH // NKV)
                    pt = work.tile([P, 10, 256], BF16, tag="pt")
                    # scores (transposed: keys on partitions) in chunks of 2 kb
                    for ci in range(0, nkb, 2):
                        ps = psp.tile([P, 512], F32, tag="ps")
                        for i in range(2):
                            kb = kbs[ci + i]
                            nc.tensor.matmul(
                                ps[:, i * 256:(i + 1) * 256],
                                kT[kv][:, kb * P:(kb + 1) * P],
                                qth[h][:],
                                start=True, stop=True,
                            )
                        nc.scalar.activation(
                            pt[:, ci:ci + 2, :], ps[:], EXP, scale=SCALE)
                    # window masks on boundary blocks
                    for i, kb in enumerate(kbs):
                        kind = None
                        if kb == 2 * t:
                            kind = 0
                        elif kb == 2 * t + 1:
                            kind = 1
                        elif kb == 2 * t - 8:
                            kind = 2
                        elif kb == 2 * t - 7:
                            kind = 3
                        if kind is not None:
                            sl = pt[:, i, :]
                            nc.vector.tensor_mul(sl, sl, mk[kind][:])
                    # denominator (ones matmul) + PV, accumulated over kbs
                    pzt = pzp.tile([P, 256], F32, tag="pz", name="pzt")
                    sut = pbp.tile([1, 256], F32, tag="su", name="sut")
                    pz = pzt[:, :]
                    su = sut[:, :]
                    for i, kb in enumerate(kbs):
                        st, sp = (i == 0), (i == nkb - 1)
                        nc.tensor.matmul(
                            su, ones[:], pt[:, i, :], start=st, stop=sp)
                        nc.tensor.matmul(
                            pz, vt[kv][kb][:], pt[:, i, :], start=st, stop=sp)
                    # normalize: bcast sums across partitions (K=1 matmul),
                    # full-lane approx reciprocal, multiply. ACT-copies free
                    # the pz/su banks fast so the next head's chains can start.
                    sus = work.tile([1, 256], BF16, tag="sus")
                    nc.scalar.copy(sus[:], su)
                    pzc = work.tile([P, 256], F32, tag="pzc")
                    nc.scalar.copy(pzc[:], pz)
                    bcps = psp.tile([P, 512], F32, tag="ps")
                    nc.tensor.matmul(bcps[:, 0:256], onesrow[:], sus[:],
                                     start=True, stop=True)
                    rec = work.tile([P, 256], F32, tag="rec")
                    nc.vector.reciprocal_approx_fast(rec[:], bcps[:, 0:256])
                    zev = zevpool.tile([P, 256], BF16, tag=f"zev{h}", name=f"zev{h}")
                    nc.vector.tensor_mul(zev[:], pzc[:], rec[:])
                    zev_cur.append(zev)
                zev_prev = zev_cur
            oproj(NS - 1, zev_prev)

    nc.compile()
    return nc


def _host_tables():
    freqs = 1.0 / (THETA ** (np.arange(0, DH - 1, 2, dtype=np.float64) / DH))
    ang = np.arange(L, dtype=np.float64)[:, None] * freqs[None, :]  # (L, 64)
    cos = np.cos(ang)
    sin = np.sin(ang)
    cosT = np.empty((P, L), np.float32)
    sinT = np.empty((P, L), np.float32)
    cosT[0::2, :] = cos.T
    cosT[1::2, :] = cos.T
    sinT[0::2, :] = -sin.T
    sinT[1::2, :] = sin.T
    return cosT, sinT


def _host_masks():
    import ml_dtypes
    k = np.arange(P)[:, None]
    q = np.arange(256)[None, :]
    m = np.stack([
        (k <= q), (k <= q - 128), (k >= q + 1), (k >= q - 127),
    ]).astype(ml_dtypes.bfloat16)
    return m.reshape(4 * P, 256)


def _pack_core_inputs(x, Wq, Wk, Wv, Wo, n, g):
    """Prepacked per-core inputs (bf16); long contiguous per-partition runs."""
    import ml_dtypes
    BF = ml_dtypes.bfloat16
    xT = np.ascontiguousarray(x[n].T)                      # (E, L)
    # xq[p, t*NKT*256 + kt*256 + c] = xT[kt*128+p, 256t + c]
    xqs = xT.reshape(NKT, P, NS, 256).transpose(1, 2, 0, 3).reshape(P, NS * NKT * 256)
    # wqp[h*128+p, kt*128+c] = Wq.T[kt*128+p, g*1024+h*128+c]
    wqT = Wq[g * 1024:(g + 1) * 1024, :].T                 # (E, 1024)
    wqp = wqT.reshape(NKT, P, NH, DH).transpose(2, 1, 0, 3).reshape(NH * P, NKT * DH)
    # wkv[p, kt*512+j]: j<256 -> Wk.T slice, j>=256 -> Wv.T slice
    wkT = Wk[g * 256:(g + 1) * 256, :].T.reshape(NKT, P, 256)
    wvT = Wv[g * 256:(g + 1) * 256, :].T.reshape(NKT, P, 256)
    wkvp = np.concatenate([wkT, wvT], axis=2)              # (kt, p, 512)
    wkvp = wkvp.transpose(1, 0, 2).reshape(P, NKT * 512)
    woT = Wo[:, g * 1024:(g + 1) * 1024].T                 # (1024, E)
    return {
        "xq": np.ascontiguousarray(xqs).astype(BF),
        "wqp": np.ascontiguousarray(wqp).astype(BF),
        "wkv": np.ascontiguousarray(wkvp).astype(BF),
        "woT": np.ascontiguousarray(woT).astype(BF),
    }


def make_in_maps(inputs):
    x = np.asarray(inputs["x"], np.float32)
    Wq = np.asarray(inputs["Wq"], np.float32)
    Wk = np.asarray(inputs["Wk"], np.float32)
    Wv = np.asarray(inputs["Wv"], np.float32)
    Wo = np.asarray(inputs["Wo"], np.float32)
    cosT, sinT = _host_tables()
    masks = _host_masks()
    in_maps = []
    for c in range(8):
        n, g = c % 4, c // 4
        m = _pack_core_inputs(x, Wq, Wk, Wv, Wo, n, g)
        m.update({"cosT": cosT, "sinT": sinT, "masks": masks})
        in_maps.append(m)
    return in_maps


def kernel(x, Wq, Wk, Wv, Wo):
    global _NC
    if _NC is None:
        _NC = build_nc()
    nc = _NC

    in_maps = make_in_maps(dict(x=x, Wq=Wq, Wk=Wk, Wv=Wv, Wo=Wo))
    from concourse.bass_utils import run_bass_kernel_spmd
    res = run_bass_kernel_spmd(nc, in_maps, list(range(8)), trace=False)
    out = np.empty((N, L, E), np.float32)
    for n_ in range(4):
        out[n_] = res.results[n_]["out"] + res.results[4 + n_]["out"]
    return out


if __name__ == "__main__":
    rng = np.random.default_rng(0)
    x = rng.standard_normal((N, L, E), dtype=np.float32)
    Wq = (rng.standard_normal((E, E), dtype=np.float32) * 0.02)
    Wk = (rng.standard_normal((E // D, E), dtype=np.float32) * 0.02)
    Wv = (rng.standard_normal((E // D, E), dtype=np.float32) * 0.02)
    Wo = (rng.standard_normal((E, E), dtype=np.float32) * 0.02)
    print(kernel(x, Wq, Wk, Wv, Wo).shape)
